# revision 7
# baseline (speedup 1.0000x reference)
"""TopK sparse autoencoder forward on 8 TRN2 NeuronCores.

Strategy: pure data-parallel over the batch (1024 rows/core, no
collectives). Per core, 4 superpasses of 256 rows (2 tiles of 128):
  encode : pre_acts = xT.T @ W_enc  (bf16 hi/lo 3-matmul split, f32
           PSUM), f32 pre_acts tile [128, 16384] held in SBUF
  topk   : per-128-window top-8 candidates (InstMax) -> 13-round
           max/match_replace cascade on the 1024 candidates -> per-row
           threshold t = 100th largest value
  mask   : encoded = pre_acts * (pre_acts >= t), written as bf16 into a
           bitcast overlay of the pre_acts tile (chunked so transposes
           can start early)
  transp : PE transposes of 128x128 blocks -> encodedT (second half of
           the overlay)
  decode : x_hat = encodedT.T @ W_dec (bf16), PSUM accumulated over
           d_sae, DMA'd straight to DRAM

All weight/activation streams use host-pre-tiled contiguous layouts so
every DMA slab is a single large contiguous read (max HBM efficiency).
"""

import numpy as np
import ml_dtypes

import concourse.mybir as mybir
from concourse.bass import Bass
from concourse.bass_utils import run_bass_kernel_spmd

import bass_rust
from concourse.tile import TileContext, ScopedClock

# This walrus build rejects instructions carrying more than a couple of
# sem waits ("Too many sync wait commands"), which Tile's scheduler and
# tail drain freely emit. Two workarounds:
#   1. PatchedTileContext re-emits the tail-drain waits as single-wait
#      sync.wait_ge instructions.
#   2. fix_sync_waits() walks the finished module and hoists excess
#      waits off any instruction onto same-engine NOPs inserted before
#      it (program order per engine is preserved).
MAX_WAITS = 1


class PatchedTileContext(TileContext):
    def _drain_and_barrier(self, tick_clock, wait_clock):
        probe = self.nc.sync.drain()
        wait_clock.add_sem_waits(
            probe.ins, ScopedClock({None: tick_clock.global_clock})
        )
        waits = list(probe.ins.sync_info.on_wait)
        probe.ins.sync_info = bass_rust.SyncInfo(on_wait=[], on_update=[])
        assert self.sems is not None
        handles = {h.num: h for h in self.sems.allocated().values()}
        for w in waits:
            sem = handles.get(w.id)
            assert sem is not None, f"no handle for sem {w.ant_name}"
            assert w.wait_mode == "sem-ge-imm", w.wait_mode
            self.nc.sync.wait_ge(sem, w.wait_value)
        self.nc.all_engine_barrier()
        popped = self.nc._tile_sem_poison_stack.pop()
        assert popped is self._sem_poison
        self.nc.clear_and_free_semaphores(list(self.sems.allocated().values()))
        self.nc.all_engine_barrier()


def fix_sync_waits(nc, max_waits=MAX_WAITS):
    ctr = 0
    for f in nc.m.functions:
        for bb in f.blocks:
            insts = list(bb.instructions)
            out, changed = [], False
            for inst in insts:
                si = inst.sync_info
                if si is not None and len(si.on_wait) > max_waits:
                    waits = list(si.on_wait)
                    head, tail = waits[:-max_waits], waits[-max_waits:]
                    for j in range(0, len(head), max_waits):
                        nop = mybir.InstNoOp(
                            name=f"I-waitfix-{ctr}", engine=inst.engine)
                        ctr += 1
                        nop.sync_info = bass_rust.SyncInfo(
                            on_wait=head[j:j + max_waits], on_update=[])
                        out.append(nop)
                    inst.sync_info = bass_rust.SyncInfo(
                        on_wait=tail, on_update=list(si.on_update))
                    changed = True
                out.append(inst)
            if changed:
                bb.instructions = out


F32 = mybir.dt.float32
BF16 = mybir.dt.bfloat16

D_IN = 2048
D_SAE = 16384
K_TOP = 100
B = 8192
N_CORES = 8
ROWS_PER_CORE = B // N_CORES      # 1024
SP_ROWS = 256                     # rows per superpass
N_SP = ROWS_PER_CORE // SP_ROWS   # 4
NTILE = SP_ROWS // 128            # 2 tiles of 128 rows
JG = 256                          # encode d_sae group width
N_JG = D_SAE // JG                # 64
WIN = 128                         # L1 window width
N_WIN = D_SAE // WIN              # 128
NCAND = 8 * N_WIN                 # 1024
NR = 13                           # cascade rounds: 13*8 = 104 >= 100
DS = 256                          # decode d_sae slab rows
N_DS = D_SAE // DS                # 64
MCH = 4                           # mask/transpose chunks per tile
NEG = -1.0e30


def build_nc(kch: int) -> Bass:
    """kch: number of 128-row contraction chunks (16, or 17 with bias row)."""
    nc = Bass()
    # all weight/x streams are host-pre-tiled: contiguous [p, c, n] slabs
    xth = nc.declare_dram_parameter("xth", [N_SP * 128, kch, SP_ROWS], BF16,
                                    isOutput=False)
    xtl = nc.declare_dram_parameter("xtl", [N_SP * 128, kch, SP_ROWS], BF16,
                                    isOutput=False)
    weh = nc.declare_dram_parameter("weh", [N_JG * 128, kch, JG], BF16,
                                    isOutput=False)
    wel = nc.declare_dram_parameter("wel", [N_JG * 128, kch, JG], BF16,
                                    isOutput=False)
    wd = nc.declare_dram_parameter("wd", [2 * N_DS * 128, DS // 128, 1024],
                                   BF16, isOutput=False)
    ident = nc.declare_dram_parameter("ident", [128, 128], BF16, isOutput=False)
    y = nc.declare_dram_parameter("y", [ROWS_PER_CORE, D_IN], F32, isOutput=True)

    with PatchedTileContext(nc) as tc:
        with (
            tc.tile_pool(name="pa", bufs=2) as pa_pool,
            tc.tile_pool(name="wep", bufs=2) as we_pool,
            tc.tile_pool(name="wepl", bufs=2) as wel_pool,
            tc.tile_pool(name="wdp", bufs=4) as wd_pool,
            tc.tile_pool(name="xtp", bufs=1) as xt_pool,
            tc.tile_pool(name="cand", bufs=1) as cand_pool,
            tc.tile_pool(name="m8", bufs=2) as m8_pool,
            tc.tile_pool(name="const", bufs=1) as const_pool,
            tc.tile_pool(name="outp", bufs=1) as out_pool,
            tc.tile_pool(name="pse", bufs=2, space="PSUM") as psum_e,
            tc.tile_pool(name="pst", bufs=2, space="PSUM") as psum_t,
            tc.tile_pool(name="psd", bufs=2, space="PSUM") as psum_d,
        ):
            identity = const_pool.tile([128, 128], BF16, name="identity")
            nc.sync.dma_start(out=identity, in_=ident[:, :])

            for sp in range(N_SP):
                r0 = sp * SP_ROWS
                xtsh = xt_pool.tile([128, kch, SP_ROWS], BF16, tag="xtsh",
                                    name="xtsh")
                xtsl = xt_pool.tile([128, kch, SP_ROWS], BF16, tag="xtsl",
                                    name="xtsl")
                kq = max(1, kch // 4)
                for q0 in range(0, kch, kq):
                    q1 = min(q0 + kq, kch)
                    nc.sync.dma_start(
                        out=xtsh[:, q0:q1, :],
                        in_=xth[sp * 128:(sp + 1) * 128, q0:q1, :])
                    nc.sync.dma_start(
                        out=xtsl[:, q0:q1, :],
                        in_=xtl[sp * 128:(sp + 1) * 128, q0:q1, :])
                pa = [pa_pool.tile([128, D_SAE], F32, tag="pa", name="pa")
                      for _ in range(NTILE)]
                cands = [cand_pool.tile([128, NCAND], F32, tag="cand",
                                        name="cand") for _ in range(NTILE)]

                m8a = [m8_pool.tile([128, NR * 8], F32, tag=f"m8a{b}",
                                    name="m8a") for b in range(NTILE)]
                # ---------- encode (hi/lo split: x@W = xh@Wh + xh@Wl + xl@Wh,
                # bf16 products are exact in f32 PSUM; residual ~2^-18) ------
                for jg in range(N_JG):
                    wesh = we_pool.tile([128, kch, JG], BF16, tag="wesh",
                                        name="wesh")
                    wesl = wel_pool.tile([128, kch, JG], BF16, tag="wesl",
                                         name="wesl")
                    kh = kch // 2
                    for q0, q1 in ((0, kh), (kh, kch)):
                        nc.sync.dma_start(
                            out=wesh[:, q0:q1, :],
                            in_=weh[jg * 128:(jg + 1) * 128, q0:q1, :])
                        nc.sync.dma_start(
                            out=wesl[:, q0:q1, :],
                            in_=wel[jg * 128:(jg + 1) * 128, q0:q1, :])
                    for b in range(NTILE):
                        ps = psum_e.tile([128, JG], F32, tag="pse", name="pse")
                        # same stationary xh[k] serves two moving operands
                        for k in range(kch):
                            xh = xtsh[:, k, b * 128:(b + 1) * 128]
                            xl = xtsl[:, k, b * 128:(b + 1) * 128]
                            nc.tensor.matmul(ps, lhsT=xh, rhs=wesh[:, k, :],
                                             start=(k == 0), stop=False)
                            nc.tensor.matmul(ps, lhsT=xh, rhs=wesl[:, k, :],
                                             start=False, stop=False)
                            nc.tensor.matmul(ps, lhsT=xl, rhs=wesh[:, k, :],
                                             start=False, stop=(k == kch - 1))
                        nc.scalar.copy(out=pa[b][:, jg * JG:(jg + 1) * JG],
                                       in_=ps)
                        # L1: top-8 of each 128-wide window, as data arrives
                        for w in range(JG // WIN):
                            wg = jg * (JG // WIN) + w
                            nc.vector.max(
                                out=cands[b][:, wg * 8:(wg + 1) * 8],
                                in_=pa[b][:, wg * WIN:(wg + 1) * WIN],
                            )
                    if jg == N_JG // 2 - 1:
                        # phase-1 cascade on the lower-half candidates runs
                        # on DVE while the PE encodes the upper half
                        for b2 in range(NTILE):
                            cur = cands[b2][:, :NCAND // 2]
                            for r in range(NR):
                                nc.vector.max(
                                    out=m8a[b2][:, r * 8:(r + 1) * 8],
                                    in_=cur)
                                if r < NR - 1:
                                    nc.vector.match_replace(
                                        out=cur,
                                        in_to_replace=m8a[b2][:,
                                                              r * 8:(r + 1) * 8],
                                        in_values=cur,
                                        imm_value=NEG,
                                    )

                # ---------- topk threshold (phase 2: cascade on the upper
                # half of the candidates, then merge with the mid-encode
                # lower-half result) ----
                t_aps = []
                for b in range(NTILE):
                    m8b = m8_pool.tile([128, NR * 8], F32, tag="m8b",
                                       name="m8b")
                    cur = cands[b][:, NCAND // 2:]
                    for r in range(NR):
                        nc.vector.max(out=m8b[:, r * 8:(r + 1) * 8], in_=cur)
                        if r < NR - 1:
                            nc.vector.match_replace(
                                out=cur,
                                in_to_replace=m8b[:, r * 8:(r + 1) * 8],
                                in_values=cur,
                                imm_value=NEG,
                            )
                    mg = m8_pool.tile([128, 2 * NR * 8], F32, tag="mg",
                                      name="mg")
                    nc.vector.tensor_copy(mg[:, :NR * 8], m8a[b])
                    nc.vector.tensor_copy(mg[:, NR * 8:], m8b)
                    m8f = m8_pool.tile([128, NR * 8], F32, tag="m8f",
                                       name="m8f")
                    for r in range(NR):
                        nc.vector.max(out=m8f[:, r * 8:(r + 1) * 8], in_=mg)
                        if r < NR - 1:
                            nc.vector.match_replace(
                                out=mg,
                                in_to_replace=m8f[:, r * 8:(r + 1) * 8],
                                in_values=mg,
                                imm_value=NEG,
                            )
                    t_aps.append(m8f[:, K_TOP - 1:K_TOP])  # 100th largest

                # ---------- mask + in-place chunk transpose ----------------
                # mask chunk mc writes bf16 enc into bytes [8192*mc,
                # 8192*mc+8191] of pa, strictly behind its own f32 read
                # region; each 128-col block is then PE-transposed and the
                # ACT copy writes the transposed block back into the SAME
                # bytes the block occupied (fully consumed by the transpose
                # read).  All cross-instruction byte ranges are disjoint, so
                # no reliance on bitcast-view alias tracking.
                enc_views = []
                for b in range(NTILE):
                    pview = pa[b].bitcast(BF16)  # [128, 32768] bf16 view
                    enc_views.append(pview)
                CW = D_SAE // MCH  # mask chunk width
                for b in range(NTILE):
                    pview = enc_views[b]
                    enc = pview[:, :D_SAE]
                    for mc in range(MCH):
                        lo = mc * CW
                        # mask chunk: encoded = pre * (pre >= t), bf16 overlay
                        nc.vector.scalar_tensor_tensor(
                            out=enc[:, lo:lo + CW],
                            in0=pa[b][:, lo:lo + CW],
                            scalar=t_aps[b],
                            in1=pa[b][:, lo:lo + CW],
                            op0=mybir.AluOpType.is_ge,
                            op1=mybir.AluOpType.mult,
                        )
                        # PE-transpose each 128x128 block back in place
                        for c in range(lo // 128, (lo + CW) // 128):
                            pt = psum_t.tile([128, 128], BF16, tag="pst",
                                             name="pst")
                            nc.tensor.transpose(
                                out=pt, in_=enc[:, c * 128:(c + 1) * 128],
                                identity=identity,
                            )
                            nc.scalar.copy(
                                out=pview[:, c * 128:(c + 1) * 128],
                                in_=pt,
                            )

                # ---------- decode ----------
                for h in range(2):  # d_in halves
                    pd = [psum_d.tile([128, 1024], F32, tag="psd", name="psd")
                          for _ in range(NTILE)]
                    for ds in range(N_DS):
                        wds = wd_pool.tile([128, DS // 128, 1024], BF16,
                                           tag="wds", name="wds")
                        nc.sync.dma_start(out=wds, in_=wd[(h * N_DS + ds) * 128:(h * N_DS + ds + 1) * 128])
                        for b in range(NTILE):
                            for c in range(DS // 128):
                                kc = ds * (DS // 128) + c
                                lhsT = enc_views[b][
                                    :, kc * 128:(kc + 1) * 128]
                                for n in range(2):
                                    nc.tensor.matmul(
                                        pd[b][:, n * 512:(n + 1) * 512],
                                        lhsT=lhsT,
                                        rhs=wds[:, c, n * 512:(n + 1) * 512],
                                        start=(kc == 0),
                                        stop=(kc == D_SAE // 128 - 1),
                                    )
                    for b in range(NTILE):
                        osb = out_pool.tile([128, 1024], F32, tag="osb",
                                            name="osb")
                        nc.scalar.copy(out=osb, in_=pd[b])
                        nc.sync.dma_start(
                            out=y[r0 + b * 128:r0 + (b + 1) * 128,
                                  h * 1024:(h + 1) * 1024],
                            in_=osb,
                        )
    return nc


def _tile_kp(a, kch, n):
    """[kch*128, n] -> contiguous [128, kch, n]."""
    return np.ascontiguousarray(
        a.reshape(kch, 128, n).transpose(1, 0, 2))


def _prep_inputs(x, W_enc, b_enc, W_dec, b_dec):
    x_eff = x - b_dec[None, :]
    if np.any(b_enc != 0.0):
        kch = D_IN // 128 + 1
        pad = kch * 128 - D_IN - 1
        we_np = np.concatenate(
            [W_enc, b_enc[None, :], np.zeros((pad, D_SAE), np.float32)], axis=0)
        x_ext = np.concatenate(
            [x_eff, np.ones((B, 1), np.float32), np.zeros((B, pad), np.float32)],
            axis=1)
    else:
        kch = D_IN // 128
        we_np = W_enc
        x_ext = x_eff
    weh_f = we_np.astype(ml_dtypes.bfloat16)
    wel_f = (we_np - weh_f.astype(np.float32)).astype(ml_dtypes.bfloat16)
    # tiled weight layouts: [N_JG, 128, kch, JG]
    weh_t = np.stack([_tile_kp(weh_f[:, j * JG:(j + 1) * JG], kch, JG)
                      for j in range(N_JG)]).reshape(N_JG * 128, kch, JG)
    wel_t = np.stack([_tile_kp(wel_f[:, j * JG:(j + 1) * JG], kch, JG)
                      for j in range(N_JG)]).reshape(N_JG * 128, kch, JG)
    wd_bf = W_dec.astype(ml_dtypes.bfloat16)
    # [2, N_DS, 128, DS//128, 1024]
    wd_t = np.stack([
        np.stack([_tile_kp(wd_bf[d * DS:(d + 1) * DS,
                                 h * 1024:(h + 1) * 1024], DS // 128, 1024)
                  for d in range(N_DS)])
        for h in range(2)]).reshape(2 * N_DS * 128, DS // 128, 1024)
    ident = np.eye(128, dtype=ml_dtypes.bfloat16)
    in_maps = []
    for i in range(N_CORES):
        rows = x_ext[i * ROWS_PER_CORE:(i + 1) * ROWS_PER_CORE]
        xt_np = np.ascontiguousarray(rows.T)  # [kch*128, 1024]
        xth_np = xt_np.astype(ml_dtypes.bfloat16)
        xtl_np = (xt_np - xth_np.astype(np.float32)).astype(ml_dtypes.bfloat16)
        # [N_SP, 128, kch, SP_ROWS]
        xth_t = np.stack([_tile_kp(xth_np[:, s * SP_ROWS:(s + 1) * SP_ROWS],
                                   kch, SP_ROWS)
                          for s in range(N_SP)]).reshape(N_SP * 128, kch,
                                                         SP_ROWS)
        xtl_t = np.stack([_tile_kp(xtl_np[:, s * SP_ROWS:(s + 1) * SP_ROWS],
                                   kch, SP_ROWS)
                          for s in range(N_SP)]).reshape(N_SP * 128, kch,
                                                         SP_ROWS)
        in_maps.append({"xth": xth_t, "xtl": xtl_t, "weh": weh_t,
                        "wel": wel_t, "wd": wd_t, "ident": ident})
    return kch, in_maps


LAST_RES = None


def kernel(x, W_enc, b_enc, W_dec, b_dec):
    global LAST_RES
    import os
    x = np.asarray(x, np.float32)
    W_enc = np.asarray(W_enc, np.float32)
    b_enc = np.asarray(b_enc, np.float32)
    W_dec = np.asarray(W_dec, np.float32)
    b_dec = np.asarray(b_dec, np.float32)
    kch, in_maps = _prep_inputs(x, W_enc, b_enc, W_dec, b_dec)
    nc = build_nc(kch)
    fix_sync_waits(nc)
    kw = {}
    if os.environ.get("KERNEL_TRACE"):
        kw = dict(trace=True, tmpdir=os.environ.get("KERNEL_TRACE_DIR"))
    res = run_bass_kernel_spmd(nc, in_maps, list(range(N_CORES)), **kw)
    LAST_RES = res
    out = np.concatenate([res.results[i]["y"] for i in range(N_CORES)], axis=0)
    if np.any(b_dec != 0.0):
        out = out + b_dec[None, :]
    return out


# revision 9
# speedup vs baseline: 1.0239x; 1.0239x over previous
"""TopK sparse autoencoder forward on 8 TRN2 NeuronCores.

Strategy: pure data-parallel over the batch (1024 rows/core, no
collectives). Per core, 4 superpasses of 256 rows (2 tiles of 128):
  encode : pre_acts = xT.T @ W_enc  (bf16 hi/lo 3-matmul split, f32
           PSUM), f32 pre_acts tile [128, 16384] held in SBUF
  topk   : per-128-window top-8 candidates (InstMax) -> 13-round
           max/match_replace cascade on the 1024 candidates -> per-row
           threshold t = 100th largest value
  mask   : encoded = pre_acts * (pre_acts >= t), written as bf16 into a
           bitcast overlay of the pre_acts tile (chunked so transposes
           can start early)
  transp : PE transposes of 128x128 blocks -> encodedT (second half of
           the overlay)
  decode : x_hat = encodedT.T @ W_dec (bf16), PSUM accumulated over
           d_sae, DMA'd straight to DRAM

All weight/activation streams use host-pre-tiled contiguous layouts so
every DMA slab is a single large contiguous read (max HBM efficiency).
"""

import numpy as np
import ml_dtypes

import concourse.mybir as mybir
from concourse.bass import Bass
from concourse.bass_utils import run_bass_kernel_spmd

import bass_rust
from concourse.tile import TileContext, ScopedClock

# This walrus build rejects instructions carrying more than a couple of
# sem waits ("Too many sync wait commands"), which Tile's scheduler and
# tail drain freely emit. Two workarounds:
#   1. PatchedTileContext re-emits the tail-drain waits as single-wait
#      sync.wait_ge instructions.
#   2. fix_sync_waits() walks the finished module and hoists excess
#      waits off any instruction onto same-engine NOPs inserted before
#      it (program order per engine is preserved).
MAX_WAITS = 1


class PatchedTileContext(TileContext):
    def _drain_and_barrier(self, tick_clock, wait_clock):
        probe = self.nc.sync.drain()
        wait_clock.add_sem_waits(
            probe.ins, ScopedClock({None: tick_clock.global_clock})
        )
        waits = list(probe.ins.sync_info.on_wait)
        probe.ins.sync_info = bass_rust.SyncInfo(on_wait=[], on_update=[])
        assert self.sems is not None
        handles = {h.num: h for h in self.sems.allocated().values()}
        for w in waits:
            sem = handles.get(w.id)
            assert sem is not None, f"no handle for sem {w.ant_name}"
            assert w.wait_mode == "sem-ge-imm", w.wait_mode
            self.nc.sync.wait_ge(sem, w.wait_value)
        self.nc.all_engine_barrier()
        popped = self.nc._tile_sem_poison_stack.pop()
        assert popped is self._sem_poison
        self.nc.clear_and_free_semaphores(list(self.sems.allocated().values()))
        self.nc.all_engine_barrier()


def fix_sync_waits(nc, max_waits=MAX_WAITS):
    ctr = 0
    for f in nc.m.functions:
        for bb in f.blocks:
            insts = list(bb.instructions)
            out, changed = [], False
            for inst in insts:
                si = inst.sync_info
                if si is not None and len(si.on_wait) > max_waits:
                    waits = list(si.on_wait)
                    head, tail = waits[:-max_waits], waits[-max_waits:]
                    for j in range(0, len(head), max_waits):
                        nop = mybir.InstNoOp(
                            name=f"I-waitfix-{ctr}", engine=inst.engine)
                        ctr += 1
                        nop.sync_info = bass_rust.SyncInfo(
                            on_wait=head[j:j + max_waits], on_update=[])
                        out.append(nop)
                    inst.sync_info = bass_rust.SyncInfo(
                        on_wait=tail, on_update=list(si.on_update))
                    changed = True
                out.append(inst)
            if changed:
                bb.instructions = out


F32 = mybir.dt.float32
BF16 = mybir.dt.bfloat16

D_IN = 2048
D_SAE = 16384
K_TOP = 100
B = 8192
N_CORES = 8
ROWS_PER_CORE = B // N_CORES      # 1024
SP_ROWS = 256                     # rows per superpass
N_SP = ROWS_PER_CORE // SP_ROWS   # 4
NTILE = SP_ROWS // 128            # 2 tiles of 128 rows
JG = 256                          # encode d_sae group width
N_JG = D_SAE // JG                # 64
WIN = 128                         # L1 window width
N_WIN = D_SAE // WIN              # 128
NCAND = 8 * N_WIN                 # 1024
NR = 13                           # cascade rounds: 13*8 = 104 >= 100
DS = 256                          # decode d_sae slab rows
N_DS = D_SAE // DS                # 64
MCH = 4                           # mask/transpose chunks per tile
NEG = -1.0e30


def build_nc(kch: int) -> Bass:
    """kch: number of 128-row contraction chunks (16, or 17 with bias row)."""
    nc = Bass()
    # all weight/x streams are host-pre-tiled: contiguous [p, c, n] slabs
    xth = nc.declare_dram_parameter("xth", [N_SP * 128, kch, SP_ROWS], BF16,
                                    isOutput=False)
    xtl = nc.declare_dram_parameter("xtl", [N_SP * 128, kch, SP_ROWS], BF16,
                                    isOutput=False)
    weh = nc.declare_dram_parameter("weh", [N_JG * 128, kch, JG], BF16,
                                    isOutput=False)
    wel = nc.declare_dram_parameter("wel", [N_JG * 128, kch, JG], BF16,
                                    isOutput=False)
    wd = nc.declare_dram_parameter("wd", [2 * N_DS * 128, DS // 128, 1024],
                                   BF16, isOutput=False)
    ident = nc.declare_dram_parameter("ident", [128, 128], BF16, isOutput=False)
    y = nc.declare_dram_parameter("y", [ROWS_PER_CORE, D_IN], F32, isOutput=True)

    with PatchedTileContext(nc) as tc:
        with (
            tc.tile_pool(name="pa", bufs=2) as pa_pool,
            tc.tile_pool(name="wep", bufs=2) as we_pool,
            tc.tile_pool(name="wepl", bufs=2) as wel_pool,
            tc.tile_pool(name="wdp", bufs=4) as wd_pool,
            tc.tile_pool(name="xtp", bufs=1) as xt_pool,
            tc.tile_pool(name="cand", bufs=1) as cand_pool,
            tc.tile_pool(name="m8", bufs=2) as m8_pool,
            tc.tile_pool(name="const", bufs=1) as const_pool,
            tc.tile_pool(name="outp", bufs=1) as out_pool,
            tc.tile_pool(name="pse", bufs=2, space="PSUM") as psum_e,
            tc.tile_pool(name="pst", bufs=2, space="PSUM") as psum_t,
            tc.tile_pool(name="psd", bufs=2, space="PSUM") as psum_d,
        ):
            identity = const_pool.tile([128, 128], BF16, name="identity")
            nc.sync.dma_start(out=identity, in_=ident[:, :])

            for sp in range(N_SP):
                r0 = sp * SP_ROWS
                xtsh = xt_pool.tile([128, kch, SP_ROWS], BF16, tag="xtsh",
                                    name="xtsh")
                nc.sync.dma_start(out=xtsh, in_=xth[sp * 128:(sp + 1) * 128])
                xtsl = xt_pool.tile([128, kch, SP_ROWS], BF16, tag="xtsl",
                                    name="xtsl")
                nc.sync.dma_start(out=xtsl, in_=xtl[sp * 128:(sp + 1) * 128])
                pa = [pa_pool.tile([128, D_SAE], F32, tag="pa", name="pa")
                      for _ in range(NTILE)]
                cands = [cand_pool.tile([128, NCAND], F32, tag="cand",
                                        name="cand") for _ in range(NTILE)]

                # ---------- encode (hi/lo split: x@W = xh@Wh + xh@Wl + xl@Wh,
                # bf16 products are exact in f32 PSUM; residual ~2^-18) ------
                for jg in range(N_JG):
                    wesh = we_pool.tile([128, kch, JG], BF16, tag="wesh",
                                        name="wesh")
                    nc.sync.dma_start(out=wesh, in_=weh[jg * 128:(jg + 1) * 128])
                    wesl = wel_pool.tile([128, kch, JG], BF16, tag="wesl",
                                         name="wesl")
                    nc.sync.dma_start(out=wesl, in_=wel[jg * 128:(jg + 1) * 128])
                    for b in range(NTILE):
                        ps = psum_e.tile([128, JG], F32, tag="pse", name="pse")
                        # same stationary xh[k] serves two moving operands
                        for k in range(kch):
                            xh = xtsh[:, k, b * 128:(b + 1) * 128]
                            xl = xtsl[:, k, b * 128:(b + 1) * 128]
                            nc.tensor.matmul(ps, lhsT=xh, rhs=wesh[:, k, :],
                                             start=(k == 0), stop=False)
                            nc.tensor.matmul(ps, lhsT=xh, rhs=wesl[:, k, :],
                                             start=False, stop=False)
                            nc.tensor.matmul(ps, lhsT=xl, rhs=wesh[:, k, :],
                                             start=False, stop=(k == kch - 1))
                        nc.scalar.copy(out=pa[b][:, jg * JG:(jg + 1) * JG],
                                       in_=ps)
                        # L1: top-8 of each 128-wide window, as data arrives
                        for w in range(JG // WIN):
                            wg = jg * (JG // WIN) + w
                            nc.vector.max(
                                out=cands[b][:, wg * 8:(wg + 1) * 8],
                                in_=pa[b][:, wg * WIN:(wg + 1) * WIN],
                            )

                # ---------- topk threshold (L2 cascade) ----
                t_aps = []
                for b in range(NTILE):
                    m8 = m8_pool.tile([128, NR * 8], F32, tag="m8", name="m8")
                    cur = cands[b]
                    for r in range(NR):
                        nc.vector.max(out=m8[:, r * 8:(r + 1) * 8], in_=cur)
                        if r < NR - 1:
                            nc.vector.match_replace(
                                out=cur,
                                in_to_replace=m8[:, r * 8:(r + 1) * 8],
                                in_values=cur,
                                imm_value=NEG,
                            )
                    t_aps.append(m8[:, K_TOP - 1:K_TOP])  # 100th largest

                # ---------- mask + in-place chunk transpose ----------------
                # mask chunk mc writes bf16 enc into bytes [8192*mc,
                # 8192*mc+8191] of pa, strictly behind its own f32 read
                # region; each 128-col block is then PE-transposed and the
                # ACT copy writes the transposed block back into the SAME
                # bytes the block occupied (fully consumed by the transpose
                # read).  All cross-instruction byte ranges are disjoint, so
                # no reliance on bitcast-view alias tracking.
                enc_views = []
                for b in range(NTILE):
                    pview = pa[b].bitcast(BF16)  # [128, 32768] bf16 view
                    enc_views.append(pview)
                CW = D_SAE // MCH  # mask chunk width
                for b in range(NTILE):
                    pview = enc_views[b]
                    enc = pview[:, :D_SAE]
                    for mc in range(MCH):
                        lo = mc * CW
                        # mask chunk: encoded = pre * (pre >= t), bf16 overlay
                        nc.vector.scalar_tensor_tensor(
                            out=enc[:, lo:lo + CW],
                            in0=pa[b][:, lo:lo + CW],
                            scalar=t_aps[b],
                            in1=pa[b][:, lo:lo + CW],
                            op0=mybir.AluOpType.is_ge,
                            op1=mybir.AluOpType.mult,
                        )
                        # PE-transpose each 128x128 block back in place
                        for c in range(lo // 128, (lo + CW) // 128):
                            pt = psum_t.tile([128, 128], BF16, tag="pst",
                                             name="pst")
                            nc.tensor.transpose(
                                out=pt, in_=enc[:, c * 128:(c + 1) * 128],
                                identity=identity,
                            )
                            nc.scalar.copy(
                                out=pview[:, c * 128:(c + 1) * 128],
                                in_=pt,
                            )

                # ---------- decode ----------
                for h in range(2):  # d_in halves
                    pd = [psum_d.tile([128, 1024], F32, tag="psd", name="psd")
                          for _ in range(NTILE)]
                    for ds in range(N_DS):
                        wds = wd_pool.tile([128, DS // 128, 1024], BF16,
                                           tag="wds", name="wds")
                        nc.sync.dma_start(out=wds, in_=wd[(h * N_DS + ds) * 128:(h * N_DS + ds + 1) * 128])
                        for b in range(NTILE):
                            for c in range(DS // 128):
                                kc = ds * (DS // 128) + c
                                lhsT = enc_views[b][
                                    :, kc * 128:(kc + 1) * 128]
                                for n in range(2):
                                    nc.tensor.matmul(
                                        pd[b][:, n * 512:(n + 1) * 512],
                                        lhsT=lhsT,
                                        rhs=wds[:, c, n * 512:(n + 1) * 512],
                                        start=(kc == 0),
                                        stop=(kc == D_SAE // 128 - 1),
                                    )
                    for b in range(NTILE):
                        osb = out_pool.tile([128, 1024], F32, tag="osb",
                                            name="osb")
                        nc.scalar.copy(out=osb, in_=pd[b])
                        nc.sync.dma_start(
                            out=y[r0 + b * 128:r0 + (b + 1) * 128,
                                  h * 1024:(h + 1) * 1024],
                            in_=osb,
                        )
    return nc


def _tile_kp(a, kch, n):
    """[kch*128, n] -> contiguous [128, kch, n]."""
    return np.ascontiguousarray(
        a.reshape(kch, 128, n).transpose(1, 0, 2))


def _prep_inputs(x, W_enc, b_enc, W_dec, b_dec):
    x_eff = x - b_dec[None, :]
    if np.any(b_enc != 0.0):
        kch = D_IN // 128 + 1
        pad = kch * 128 - D_IN - 1
        we_np = np.concatenate(
            [W_enc, b_enc[None, :], np.zeros((pad, D_SAE), np.float32)], axis=0)
        x_ext = np.concatenate(
            [x_eff, np.ones((B, 1), np.float32), np.zeros((B, pad), np.float32)],
            axis=1)
    else:
        kch = D_IN // 128
        we_np = W_enc
        x_ext = x_eff
    weh_f = we_np.astype(ml_dtypes.bfloat16)
    wel_f = (we_np - weh_f.astype(np.float32)).astype(ml_dtypes.bfloat16)
    # tiled weight layouts: [N_JG, 128, kch, JG]
    weh_t = np.stack([_tile_kp(weh_f[:, j * JG:(j + 1) * JG], kch, JG)
                      for j in range(N_JG)]).reshape(N_JG * 128, kch, JG)
    wel_t = np.stack([_tile_kp(wel_f[:, j * JG:(j + 1) * JG], kch, JG)
                      for j in range(N_JG)]).reshape(N_JG * 128, kch, JG)
    wd_bf = W_dec.astype(ml_dtypes.bfloat16)
    # [2, N_DS, 128, DS//128, 1024]
    wd_t = np.stack([
        np.stack([_tile_kp(wd_bf[d * DS:(d + 1) * DS,
                                 h * 1024:(h + 1) * 1024], DS // 128, 1024)
                  for d in range(N_DS)])
        for h in range(2)]).reshape(2 * N_DS * 128, DS // 128, 1024)
    ident = np.eye(128, dtype=ml_dtypes.bfloat16)
    in_maps = []
    for i in range(N_CORES):
        rows = x_ext[i * ROWS_PER_CORE:(i + 1) * ROWS_PER_CORE]
        xt_np = np.ascontiguousarray(rows.T)  # [kch*128, 1024]
        xth_np = xt_np.astype(ml_dtypes.bfloat16)
        xtl_np = (xt_np - xth_np.astype(np.float32)).astype(ml_dtypes.bfloat16)
        # [N_SP, 128, kch, SP_ROWS]
        xth_t = np.stack([_tile_kp(xth_np[:, s * SP_ROWS:(s + 1) * SP_ROWS],
                                   kch, SP_ROWS)
                          for s in range(N_SP)]).reshape(N_SP * 128, kch,
                                                         SP_ROWS)
        xtl_t = np.stack([_tile_kp(xtl_np[:, s * SP_ROWS:(s + 1) * SP_ROWS],
                                   kch, SP_ROWS)
                          for s in range(N_SP)]).reshape(N_SP * 128, kch,
                                                         SP_ROWS)
        in_maps.append({"xth": xth_t, "xtl": xtl_t, "weh": weh_t,
                        "wel": wel_t, "wd": wd_t, "ident": ident})
    return kch, in_maps


LAST_RES = None


def kernel(x, W_enc, b_enc, W_dec, b_dec):
    global LAST_RES
    import os
    x = np.asarray(x, np.float32)
    W_enc = np.asarray(W_enc, np.float32)
    b_enc = np.asarray(b_enc, np.float32)
    W_dec = np.asarray(W_dec, np.float32)
    b_dec = np.asarray(b_dec, np.float32)
    kch, in_maps = _prep_inputs(x, W_enc, b_enc, W_dec, b_dec)
    nc = build_nc(kch)
    fix_sync_waits(nc)
    kw = {}
    if os.environ.get("KERNEL_TRACE"):
        kw = dict(trace=True, tmpdir=os.environ.get("KERNEL_TRACE_DIR"))
    res = run_bass_kernel_spmd(nc, in_maps, list(range(N_CORES)), **kw)
    LAST_RES = res
    out = np.concatenate([res.results[i]["y"] for i in range(N_CORES)], axis=0)
    if np.any(b_dec != 0.0):
        out = out + b_dec[None, :]
    return out
